# revision 4
# baseline (speedup 1.0000x reference)
"""Trainium2 Bass kernel for nn_Attention (conv/lstm attention + softmax pooling).

reference:
    att1 = einsum("bpc,ac->bpa", conv_out, Wc) + bc          # (B, P, A)
    att2 = lstm_hidden @ Wl.T + bl                           # (B, A)
    att  = einsum("bpa,a->bp", relu(att1 + att2[:,None,:]), Wf[0]) + bf[0]
    alpha = softmax(att, axis=1)                             # over P pixels
    out  = einsum("bpc,bp->bc", conv_out, alpha)             # (B, C)

Shapes: B=256, P=196, C=2048, A=512, L=512.  8 cores, data-parallel over batch.

Kernel math (per core, 32 batches):
  - fold |Wf[a]| into Wc/Wl/biases and permute the a-axis so Wf>0 columns come
    first; then att = sum_pos relu(z') - sum_neg relu(z') where
    z' = conv@Wc'.T + (lstm@Wl'.T + bias') broadcast over pixels.
    (bf[0] shifts all att uniformly -> cancels in softmax -> dropped.)
  - big matmul over m-chunks of 128 (batch*pixel) rows; per-row bias via an
    indicator matmul accumulated into the same PSUM group.
  - relu + sum over a: two ScalarE activations with accum_out (pos/neg halves).
  - softmax: PE-transpose + DRAM bounce to [8,196] layout; exp via ACT with
    per-partition -max bias and fused accum (sum of exps). alpha stays
    unnormalized; 1/sum is applied to the final output rows instead.
  - weighted pixel sum: f32r matvecs expT.T @ conv_nat (second, natural-layout
    stream of conv_out), accumulated over the two pixel chunks (128+68).
"""

import sys

sys.path.insert(0, "/opt/trn_rl_repo")

import numpy as np
import ml_dtypes

import concourse.bass as bass
import concourse.tile as tile
from concourse import bacc, mybir
from concourse import bass_utils

f32 = mybir.dt.float32
f32r = mybir.dt.float32r
bf16 = mybir.dt.bfloat16
AF = mybir.ActivationFunctionType
AX = mybir.AxisListType
OP = mybir.AluOpType

# ---- problem constants (hardcoded; kernel.py must be self-contained) ----
BATCH, NPIX, CONV_DIM, ATT_DIM, LSTM_DIM = 256, 196, 2048, 512, 512
N_CORES = 8
B = BATCH // N_CORES          # 32 batches per core
KO = CONV_DIM // 128          # 16 contraction chunks
LK = LSTM_DIM // 128          # 4
import os
G = int(os.environ.get("K_GROUPS", "4"))  # batch groups per core
BARRIER = os.environ.get("K_BARRIER", "0") == "1"
GPOOL_BUFS = int(os.environ.get("K_GPOOL_BUFS", "2"))
GB = B // G                   # 8 batches per group
RG = GB * NPIX                # 1568 valid rows per group
MC = (RG + 127) // 128        # 13 m-chunks per group
RGP = MC * 128                # 1664 padded rows per group
NC_CHUNKS = CONV_DIM // 512   # 4 output column chunks

# dtype of the transposed conv stream / Wc (main matmul). bf16 halves DMA.
DT1 = {"bf16": bf16, "f32r": f32r}[os.environ.get("K_DT1", "bf16")]
DT1_NP = {"bf16": ml_dtypes.bfloat16, "f32r": np.float32}[os.environ.get("K_DT1", "bf16")]
# dtype of the weighted-sum path (convN stream, exp weights)
WS_DT = {"bf16": bf16, "f32r": f32r}[os.environ.get("K_WS_DT", "f32r")]


def build_program(a_pos: int, reps: int = 1):
    """Build the Bass program. a_pos = number of positive Wf entries (after
    the host-side permutation positives-first). reps>1 wraps the body in a
    hardware loop for timing."""
    nc = bacc.Bacc("TRN2", target_bir_lowering=False, debug=False,
                   num_devices=N_CORES)

    # ---- DRAM tensors ----
    convT = nc.dram_tensor("convT", [G, MC, 128, KO, 128], DT1,
                           kind="ExternalInput").ap()
    convN = nc.dram_tensor("convN", [B, NPIX, CONV_DIM], WS_DT,
                           kind="ExternalInput").ap()
    lstmT = nc.dram_tensor("lstmT", [128, LK, B], f32r, kind="ExternalInput").ap()
    wc = nc.dram_tensor("wc", [128, KO, 512], DT1, kind="ExternalInput").ap()
    wl = nc.dram_tensor("wl", [128, LK, 512], f32r, kind="ExternalInput").ap()
    biasp = nc.dram_tensor("biasp", [1, 512], f32r, kind="ExternalInput").ap()
    onesb = nc.dram_tensor("onesb", [1, B], f32r, kind="ExternalInput").ap()
    ind = nc.dram_tensor("ind", [GB, RGP], f32r, kind="ExternalInput").ap()

    att_bounce = nc.dram_tensor("att_bounce", [G, MC, 128], f32)
    e_bounce = nc.dram_tensor("e_bounce", [G, GB, NPIX], WS_DT)
    raw = nc.dram_tensor("raw", [B, CONV_DIM], f32)
    out = nc.dram_tensor("out", [B, CONV_DIM], f32, kind="ExternalOutput").ap()

    with tile.TileContext(nc) as tc:
        import contextlib
        with contextlib.ExitStack() as ctx:
            consts = ctx.enter_context(tc.tile_pool(name="consts", bufs=1))
            gpool = ctx.enter_context(tc.tile_pool(name="gpool", bufs=GPOOL_BUFS))
            mpool = ctx.enter_context(tc.tile_pool(name="mpool", bufs=3))
            cnpool = ctx.enter_context(tc.tile_pool(name="cnpool", bufs=4))
            spool = ctx.enter_context(tc.tile_pool(name="spool", bufs=2))
            psA = ctx.enter_context(tc.tile_pool(name="psA", bufs=3, space="PSUM"))
            psW = ctx.enter_context(tc.tile_pool(name="psW", bufs=2, space="PSUM"))

            # ---- resident constants ----
            wc_sb = consts.tile([128, KO, 512], DT1, tag="wc")
            nc.sync.dma_start(wc_sb[:], wc)
            wl_sb = consts.tile([128, LK, 512], f32r, tag="wl")
            nc.sync.dma_start(wl_sb[:], wl)
            lstmT_sb = consts.tile([128, LK, B], f32r, tag="lstmT")
            nc.sync.dma_start(lstmT_sb[:], lstmT)
            biasp_sb = consts.tile([1, 512], f32r, tag="biasp")
            nc.sync.dma_start(biasp_sb[:], biasp)
            onesb_sb = consts.tile([1, B], f32r, tag="onesb")
            nc.sync.dma_start(onesb_sb[:], onesb)
            ind_sb = consts.tile([GB, RGP], f32r, tag="ind")
            nc.sync.dma_start(ind_sb[:], ind)
            att2b_sb = consts.tile([GB, G, 512], f32r, tag="att2b")

            def body():
                # ---- att2b[j, g, :] = lstm[8g+j] @ Wl'.T + bias'  (f32r) ----
                for g in range(G):
                    pa = psA.tile([128, 512], f32, tag="att1")
                    for k in range(LK):
                        nc.tensor.matmul(pa[0:GB, :],
                                         lstmT_sb[:, k, g * GB:(g + 1) * GB],
                                         wl_sb[:, k, :],
                                         start=(k == 0), stop=False)
                    nc.tensor.matmul(pa[0:GB, :], onesb_sb[:, 0:GB], biasp_sb[:],
                                     start=False, stop=True)
                    nc.scalar.copy(att2b_sb[:, g, :], pa[0:GB, :])

                for g in range(G):
                    if BARRIER:
                        tc.strict_bb_all_engine_barrier()
                    att_pos = gpool.tile([128, MC], f32, tag="att_pos")
                    att_neg = gpool.tile([128, MC], f32, tag="att_neg")
                    # ---- main matmul phase over m-chunks ----
                    for mc in range(MC):
                        ct = mpool.tile([128, KO, 128], DT1, tag="ct")
                        nc.sync.dma_start(ct[:], convT[g, mc])
                        pa = psA.tile([128, 512], f32, tag="att1")
                        for k in range(KO):
                            nc.tensor.matmul(pa[:], ct[:, k, :], wc_sb[:, k, :],
                                             start=(k == 0), stop=False)
                        nc.tensor.matmul(pa[:],
                                         ind_sb[:, mc * 128:(mc + 1) * 128],
                                         att2b_sb[:, g, :],
                                         start=False, stop=True)
                        # relu + split sums over the a axis (Wf sign split)
                        scratch = spool.tile([128, 512], f32, tag="scratch")
                        if a_pos > 0:
                            nc.scalar.activation(scratch[:, :a_pos], pa[:, :a_pos],
                                                 AF.Relu,
                                                 accum_out=att_pos[:, mc:mc + 1])
                        else:
                            nc.vector.memset(att_pos[:, mc:mc + 1], 0.0)
                        if a_pos < 512:
                            nc.scalar.activation(scratch[:, a_pos:], pa[:, a_pos:],
                                                 AF.Relu,
                                                 accum_out=att_neg[:, mc:mc + 1])
                        else:
                            nc.vector.memset(att_neg[:, mc:mc + 1], 0.0)

                    # ---- softmax head: att -> [GB, 196] layout via bounce ----
                    att_sub = gpool.tile([128, MC], f32, tag="att_sub")
                    nc.vector.tensor_tensor(att_sub[:], att_pos[:], att_neg[:],
                                            OP.subtract)
                    nc.sync.dma_start(
                        att_bounce.ap()[g].rearrange("mc p -> p mc"), att_sub[:])
                    att_bp = gpool.tile([GB, NPIX], f32, tag="att_bp")
                    nc.sync.dma_start(
                        att_bp[:],
                        att_bounce.ap()[g].rearrange("a b -> (a b)")[0:RG]
                        .rearrange("(a b) -> a b", a=GB))
                    negmax = gpool.tile([GB, 1], f32, tag="negmax")
                    nc.vector.tensor_reduce(negmax[:], att_bp[:], axis=AX.X,
                                            op=OP.max, negate=True)
                    e_sb = gpool.tile([GB, NPIX], WS_DT, tag="e_sb")
                    sume = gpool.tile([GB, 1], f32, tag="sume")
                    nc.scalar.activation(e_sb[:], att_bp[:], AF.Exp,
                                         bias=negmax[:], scale=1.0,
                                         accum_out=sume[:])
                    rsum = gpool.tile([GB, 1], f32, tag="rsum")
                    nc.vector.reciprocal(rsum[:], sume[:])
                    # bounce exp through DRAM to get pixel-on-partition cols
                    nc.sync.dma_start(e_bounce.ap()[g], e_sb[:])
                    eT0 = gpool.tile([128, GB], WS_DT, tag="eT0")
                    nc.sync.dma_start(
                        eT0[:], e_bounce.ap()[g][:, 0:128].rearrange("b p -> p b"))
                    eT1 = gpool.tile([NPIX - 128, GB], WS_DT, tag="eT1")
                    nc.sync.dma_start(
                        eT1[:], e_bounce.ap()[g][:, 128:NPIX].rearrange("b p -> p b"))

                    # ---- weighted pixel sum per batch ----
                    for j in range(GB):
                        b = g * GB + j
                        cn0 = cnpool.tile([128, CONV_DIM], WS_DT, tag="cn0")
                        nc.sync.dma_start(cn0[:], convN[b, 0:128, :])
                        cn1 = cnpool.tile([NPIX - 128, CONV_DIM], WS_DT, tag="cn1")
                        nc.sync.dma_start(cn1[:], convN[b, 128:NPIX, :])
                        wsrow = spool.tile([1, CONV_DIM], f32, tag="wsrow")
                        for n in range(NC_CHUNKS):
                            pw = psW.tile([1, 512], f32, tag="ws")
                            nc.tensor.matmul(pw[:], eT0[:, j:j + 1],
                                             cn0[:, n * 512:(n + 1) * 512],
                                             start=True, stop=False)
                            nc.tensor.matmul(pw[:], eT1[:, j:j + 1],
                                             cn1[:, n * 512:(n + 1) * 512],
                                             start=False, stop=True)
                            nc.any.tensor_copy(
                                out=wsrow[:, n * 512:(n + 1) * 512], in_=pw[:])
                        nc.sync.dma_start(raw.ap()[b:b + 1, :], wsrow[:])

                    # ---- normalize group rows by 1/sumexp ----
                    rawg = gpool.tile([GB, CONV_DIM], f32, tag="rawg")
                    nc.sync.dma_start(rawg[:], raw.ap()[g * GB:(g + 1) * GB, :])
                    outg = gpool.tile([GB, CONV_DIM], f32, tag="outg")
                    nc.scalar.mul(outg[:], rawg[:], rsum[:])
                    nc.sync.dma_start(out[g * GB:(g + 1) * GB, :], outg[:])

            if reps == 1:
                body()
            else:
                with tc.For_i(0, reps, 1):
                    body()

    nc.compile()
    return nc


def host_pack(conv_out, lstm_hidden, Wc, bc, Wl, bl, Wf, bf):
    """Host-side preprocessing -> (shared dict, per-core input dicts)."""
    conv_out = np.asarray(conv_out, dtype=np.float32)
    lstm_hidden = np.asarray(lstm_hidden, dtype=np.float32)
    Wc = np.asarray(Wc, dtype=np.float32)
    bc = np.asarray(bc, dtype=np.float32)
    Wl = np.asarray(Wl, dtype=np.float32)
    bl = np.asarray(bl, dtype=np.float32)
    wf = np.asarray(Wf, dtype=np.float32)[0]
    # permute a axis: Wf>0 first; fold |Wf| into Wc/Wl/bias
    pos = np.nonzero(wf > 0)[0]
    neg = np.nonzero(wf <= 0)[0]
    perm = np.concatenate([pos, neg])
    a_pos = int(len(pos))
    absf = np.abs(wf[perm])[:, None]                       # [512, 1]
    Wcp = (np.abs(wf)[:, None] * Wc)[perm]                 # [512, 2048]
    Wlp = (np.abs(wf)[:, None] * Wl)[perm]                 # [512, 512]
    biasp = (np.abs(wf) * (bc + bl))[perm]                 # [512]

    wc_pack = np.ascontiguousarray(
        Wcp.T.reshape(KO, 128, 512).transpose(1, 0, 2)).astype(DT1_NP)
    wl_pack = np.ascontiguousarray(
        Wlp.T.reshape(LK, 128, 512).transpose(1, 0, 2))
    shared = {
        "wc": wc_pack,
        "wl": wl_pack,
        "biasp": biasp[None, :].copy(),
        "onesb": np.ones((1, B), np.float32),
    }
    # indicator: ind[j, r] = 1 if r // 196 == j (r < RG)
    indm = np.zeros((GB, RGP), np.float32)
    r = np.arange(RG)
    indm[r // NPIX, r] = 1.0
    shared["ind"] = indm

    in_maps = []
    for c in range(N_CORES):
        sl = slice(c * B, (c + 1) * B)
        conv_core = conv_out[sl]                           # [32, 196, 2048]
        flat = conv_core.reshape(B * NPIX, CONV_DIM)
        ct = np.zeros((G, MC, 128, KO, 128), DT1_NP)
        for g in range(G):
            block = flat[g * RG:(g + 1) * RG]              # [1568, 2048]
            padded = np.zeros((RGP, CONV_DIM), np.float32)
            padded[:RG] = block
            # [mc, j, ko, p] -> [mc, p, ko, j]
            ct[g] = padded.reshape(MC, 128, KO, 128).transpose(0, 3, 2, 1)
        lstm_core = lstm_hidden[sl]                        # [32, 512]
        lstmT_pack = np.ascontiguousarray(
            lstm_core.T.reshape(LK, 128, B).transpose(1, 0, 2))
        in_maps.append(dict(shared,
                            convT=ct,
                            convN=np.ascontiguousarray(conv_core).astype(
                                np.float32 if WS_DT == f32r else ml_dtypes.bfloat16),
                            lstmT=lstmT_pack))
    return in_maps, a_pos


_CACHE = {}


def _get_program(a_pos, reps=1):
    key = (a_pos, reps)
    if key not in _CACHE:
        _CACHE[key] = build_program(a_pos, reps)
    return _CACHE[key]


def kernel(**inputs) -> np.ndarray:
    in_maps, a_pos = host_pack(**inputs)
    nc = _get_program(a_pos)
    res = bass_utils.run_bass_kernel_spmd(nc, in_maps,
                                          core_ids=list(range(N_CORES)))
    return np.concatenate([r["out"] for r in res.results], axis=0)


# revision 6
# speedup vs baseline: 1.2417x; 1.2417x over previous
"""Trainium2 Bass kernel for nn_Attention (conv/lstm attention + softmax pooling).

reference:
    att1 = einsum("bpc,ac->bpa", conv_out, Wc) + bc          # (B, P, A)
    att2 = lstm_hidden @ Wl.T + bl                           # (B, A)
    att  = einsum("bpa,a->bp", relu(att1 + att2[:,None,:]), Wf[0]) + bf[0]
    alpha = softmax(att, axis=1)                             # over P pixels
    out  = einsum("bpc,bp->bc", conv_out, alpha)             # (B, C)

Shapes: B=256, P=196, C=2048, A=512, L=512.  8 cores, data-parallel over batch.

Kernel math (per core, 32 batches):
  - fold |Wf[a]| into Wc/Wl/biases and permute the a-axis so Wf>0 columns come
    first; then att = sum_pos relu(z') - sum_neg relu(z') where
    z' = conv@Wc'.T + (lstm@Wl'.T + bias') broadcast over pixels.
    (bf[0] shifts all att uniformly -> cancels in softmax -> dropped.)
  - big matmul over m-chunks of 128 (batch*pixel) rows; per-row bias via an
    indicator matmul accumulated into the same PSUM group.
  - relu + sum over a: two ScalarE activations with accum_out (pos/neg halves).
  - softmax: PE-transpose + DRAM bounce to [8,196] layout; exp via ACT with
    per-partition -max bias and fused accum (sum of exps). alpha stays
    unnormalized; 1/sum is applied to the final output rows instead.
  - weighted pixel sum: f32r matvecs expT.T @ conv_nat (second, natural-layout
    stream of conv_out), accumulated over the two pixel chunks (128+68).
"""

import sys

sys.path.insert(0, "/opt/trn_rl_repo")

import numpy as np
import ml_dtypes

import concourse.bass as bass
import concourse.tile as tile
from concourse import bacc, mybir
from concourse import bass_utils

f32 = mybir.dt.float32
f32r = mybir.dt.float32r
bf16 = mybir.dt.bfloat16
AF = mybir.ActivationFunctionType
AX = mybir.AxisListType
OP = mybir.AluOpType

# ---- problem constants (hardcoded; kernel.py must be self-contained) ----
BATCH, NPIX, CONV_DIM, ATT_DIM, LSTM_DIM = 256, 196, 2048, 512, 512
N_CORES = 8
B = BATCH // N_CORES          # 32 batches per core
KO = CONV_DIM // 128          # 16 contraction chunks
LK = LSTM_DIM // 128          # 4
import os
G = int(os.environ.get("K_GROUPS", "4"))  # batch groups per core
BARRIER = os.environ.get("K_BARRIER", "0") == "1"
GPOOL_BUFS = int(os.environ.get("K_GPOOL_BUFS", "2"))
WS_PACK = os.environ.get("K_WS_PACK", "0") == "1"
MPOOL_BUFS = int(os.environ.get("K_MPOOL_BUFS", "3"))
CN_BUFS = int(os.environ.get("K_CN_BUFS", "4"))
PSA_BUFS = int(os.environ.get("K_PSA_BUFS", "3"))
ABLATE = set(os.environ.get("K_ABLATE", "").split(",")) - {""}
GB = B // G                   # 8 batches per group
RG = GB * NPIX                # 1568 valid rows per group
MC = (RG + 127) // 128        # 13 m-chunks per group
RGP = MC * 128                # 1664 padded rows per group
NC_CHUNKS = CONV_DIM // 512   # 4 output column chunks

# dtype of the transposed conv stream / Wc (main matmul). bf16 halves DMA.
DT1 = {"bf16": bf16, "f32r": f32r}[os.environ.get("K_DT1", "bf16")]
DT1_NP = {"bf16": ml_dtypes.bfloat16, "f32r": np.float32}[os.environ.get("K_DT1", "bf16")]
# dtype of the weighted-sum path (convN stream, exp weights)
WS_DT = {"bf16": bf16, "f32r": f32r}[os.environ.get("K_WS_DT", "f32r")]


def build_program(a_pos: int, reps: int = 1):
    """Build the Bass program. a_pos = number of positive Wf entries (after
    the host-side permutation positives-first). reps>1 wraps the body in a
    hardware loop for timing."""
    nc = bacc.Bacc("TRN2", target_bir_lowering=False, debug=False,
                   num_devices=N_CORES)

    # ---- DRAM tensors ----
    convT = nc.dram_tensor("convT", [G, MC, 128, KO, 128], DT1,
                           kind="ExternalInput").ap()
    convN = nc.dram_tensor("convN", [B, NPIX, CONV_DIM], WS_DT,
                           kind="ExternalInput").ap()
    lstmT = nc.dram_tensor("lstmT", [128, LK, B], f32r, kind="ExternalInput").ap()
    wc = nc.dram_tensor("wc", [128, KO, 512], DT1, kind="ExternalInput").ap()
    wl = nc.dram_tensor("wl", [128, LK, 512], f32r, kind="ExternalInput").ap()
    biasp = nc.dram_tensor("biasp", [1, 512], f32r, kind="ExternalInput").ap()
    onesb = nc.dram_tensor("onesb", [1, B], f32r, kind="ExternalInput").ap()
    ind = nc.dram_tensor("ind", [GB, RGP], f32r, kind="ExternalInput").ap()

    att_bounce = nc.dram_tensor("att_bounce", [G, MC, 128], f32)
    e_bounce = nc.dram_tensor("e_bounce", [G, GB, NPIX], WS_DT)
    raw = nc.dram_tensor("raw", [B, CONV_DIM], f32)
    out = nc.dram_tensor("out", [B, CONV_DIM], f32, kind="ExternalOutput").ap()

    with tile.TileContext(nc) as tc:
        import contextlib
        with contextlib.ExitStack() as ctx:
            consts = ctx.enter_context(tc.tile_pool(name="consts", bufs=1))
            gpool = ctx.enter_context(tc.tile_pool(name="gpool", bufs=GPOOL_BUFS))
            mpool = ctx.enter_context(tc.tile_pool(name="mpool", bufs=MPOOL_BUFS))
            cnpool = ctx.enter_context(tc.tile_pool(name="cnpool", bufs=CN_BUFS))
            spool = ctx.enter_context(tc.tile_pool(name="spool", bufs=2))
            psA = ctx.enter_context(tc.tile_pool(name="psA", bufs=PSA_BUFS, space="PSUM"))
            psW = ctx.enter_context(tc.tile_pool(name="psW", bufs=2, space="PSUM"))

            # ---- resident constants ----
            wc_sb = consts.tile([128, KO, 512], DT1, tag="wc")
            nc.sync.dma_start(wc_sb[:], wc)
            wl_sb = consts.tile([128, LK, 512], f32r, tag="wl")
            nc.sync.dma_start(wl_sb[:], wl)
            lstmT_sb = consts.tile([128, LK, B], f32r, tag="lstmT")
            nc.sync.dma_start(lstmT_sb[:], lstmT)
            biasp_sb = consts.tile([1, 512], f32r, tag="biasp")
            nc.sync.dma_start(biasp_sb[:], biasp)
            onesb_sb = consts.tile([1, B], f32r, tag="onesb")
            nc.sync.dma_start(onesb_sb[:], onesb)
            ind_sb = consts.tile([GB, RGP], f32r, tag="ind")
            nc.sync.dma_start(ind_sb[:], ind)
            att2b_sb = consts.tile([GB, G, 512], f32r, tag="att2b")

            def body():
                # ---- att2b[j, g, :] = lstm[8g+j] @ Wl'.T + bias'  (f32r) ----
                for g in range(G):
                    pa = psA.tile([128, 512], f32, tag="att1")
                    for k in range(LK):
                        nc.tensor.matmul(pa[0:GB, :],
                                         lstmT_sb[:, k, g * GB:(g + 1) * GB],
                                         wl_sb[:, k, :],
                                         start=(k == 0), stop=False)
                    nc.tensor.matmul(pa[0:GB, :], onesb_sb[:, 0:GB], biasp_sb[:],
                                     start=False, stop=True)
                    nc.scalar.copy(att2b_sb[:, g, :], pa[0:GB, :])

                for g in range(G):
                    if BARRIER:
                        tc.strict_bb_all_engine_barrier()
                    att_pos = gpool.tile([128, MC], f32, tag="att_pos")
                    att_neg = gpool.tile([128, MC], f32, tag="att_neg")
                    # ---- main matmul phase over m-chunks ----
                    for mc in range(MC):
                        ct = mpool.tile([128, KO, 128], DT1, tag="ct")
                        nc.sync.dma_start(ct[:], convT[g, mc])
                        pa = psA.tile([128, 512], f32, tag="att1")
                        kos = range(KO) if "mm" not in ABLATE else range(1)
                        for k in kos:
                            nc.tensor.matmul(pa[:], ct[:, k, :], wc_sb[:, k, :],
                                             start=(k == 0), stop=False)
                        nc.tensor.matmul(pa[:],
                                         ind_sb[:, mc * 128:(mc + 1) * 128],
                                         att2b_sb[:, g, :],
                                         start=False, stop=True)
                        # relu + split sums over the a axis (Wf sign split)
                        scratch = spool.tile([128, 512], f32, tag="scratch")
                        if a_pos > 0:
                            nc.scalar.activation(scratch[:, :a_pos], pa[:, :a_pos],
                                                 AF.Relu,
                                                 accum_out=att_pos[:, mc:mc + 1])
                        else:
                            nc.vector.memset(att_pos[:, mc:mc + 1], 0.0)
                        if a_pos < 512:
                            nc.scalar.activation(scratch[:, a_pos:], pa[:, a_pos:],
                                                 AF.Relu,
                                                 accum_out=att_neg[:, mc:mc + 1])
                        else:
                            nc.vector.memset(att_neg[:, mc:mc + 1], 0.0)

                    # ---- softmax head: att -> [GB, 196] layout via bounce ----
                    att_sub = gpool.tile([128, MC], f32, tag="att_sub")
                    nc.vector.tensor_tensor(att_sub[:], att_pos[:], att_neg[:],
                                            OP.subtract)
                    nc.sync.dma_start(
                        att_bounce.ap()[g].rearrange("mc p -> p mc"), att_sub[:])
                    att_bp = gpool.tile([GB, NPIX], f32, tag="att_bp")
                    nc.sync.dma_start(
                        att_bp[:],
                        att_bounce.ap()[g].rearrange("a b -> (a b)")[0:RG]
                        .rearrange("(a b) -> a b", a=GB))
                    negmax = gpool.tile([GB, 1], f32, tag="negmax")
                    nc.vector.tensor_reduce(negmax[:], att_bp[:], axis=AX.X,
                                            op=OP.max, negate=True)
                    e_sb = gpool.tile([GB, NPIX], WS_DT, tag="e_sb")
                    sume = gpool.tile([GB, 1], f32, tag="sume")
                    nc.scalar.activation(e_sb[:], att_bp[:], AF.Exp,
                                         bias=negmax[:], scale=1.0,
                                         accum_out=sume[:])
                    rsum = gpool.tile([GB, 1], f32, tag="rsum")
                    nc.vector.reciprocal(rsum[:], sume[:])
                    # bounce exp through DRAM to get pixel-on-partition cols
                    nc.sync.dma_start(e_bounce.ap()[g], e_sb[:])
                    eT0 = gpool.tile([128, GB], WS_DT, tag="eT0")
                    nc.sync.dma_start(
                        eT0[:], e_bounce.ap()[g][:, 0:128].rearrange("b p -> p b"))
                    eT1 = gpool.tile([NPIX - 128, GB], WS_DT, tag="eT1")
                    nc.sync.dma_start(
                        eT1[:], e_bounce.ap()[g][:, 128:NPIX].rearrange("b p -> p b"))

                    # ---- weighted pixel sum per batch ----
                    if "ws" in ABLATE:
                        pass
                    elif WS_PACK:
                        # 4 batches per pass, packed into one PSUM bank at
                        # partitions 0/32/64/96 via col tiling -> 4x concurrency
                        for j0 in range(0, GB, 4):
                            cns = []
                            for dj in range(4):
                                b = g * GB + j0 + dj
                                cn0 = cnpool.tile([128, CONV_DIM], WS_DT, tag="cn0")
                                nc.sync.dma_start(cn0[:], convN[b, 0:128, :])
                                cn1 = cnpool.tile([NPIX - 128, CONV_DIM], WS_DT,
                                                  tag="cn1")
                                nc.sync.dma_start(cn1[:], convN[b, 128:NPIX, :])
                                cns.append((cn0, cn1))
                            wsrows = spool.tile([128, CONV_DIM], f32, tag="wsrow")
                            for n in range(NC_CHUNKS):
                                pw = psW.tile([128, 512], f32, tag="ws")
                                for dj in range(4):
                                    j = j0 + dj
                                    cn0, cn1 = cns[dj]
                                    nc.tensor.matmul(
                                        pw[32 * dj:32 * dj + 1, :],
                                        eT0[:, j:j + 1],
                                        cn0[:, n * 512:(n + 1) * 512],
                                        start=True, stop=False,
                                        tile_position=(0, 32 * dj))
                                    nc.tensor.matmul(
                                        pw[32 * dj:32 * dj + 1, :],
                                        eT1[:, j:j + 1],
                                        cn1[:, n * 512:(n + 1) * 512],
                                        start=False, stop=True,
                                        tile_position=(0, 32 * dj))
                                nc.any.tensor_copy(
                                    out=wsrows[:, n * 512:(n + 1) * 512], in_=pw[:])
                            for dj in range(4):
                                b = g * GB + j0 + dj
                                nc.sync.dma_start(
                                    raw.ap()[b:b + 1, :],
                                    wsrows[32 * dj:32 * dj + 1, :])
                    else:
                        for j in range(GB):
                            b = g * GB + j
                            cn0 = cnpool.tile([128, CONV_DIM], WS_DT, tag="cn0")
                            nc.sync.dma_start(cn0[:], convN[b, 0:128, :])
                            cn1 = cnpool.tile([NPIX - 128, CONV_DIM], WS_DT,
                                              tag="cn1")
                            nc.sync.dma_start(cn1[:], convN[b, 128:NPIX, :])
                            wsrow = spool.tile([1, CONV_DIM], f32, tag="wsrow")
                            for n in range(NC_CHUNKS):
                                pw = psW.tile([1, 512], f32, tag="ws")
                                nc.tensor.matmul(pw[:], eT0[:, j:j + 1],
                                                 cn0[:, n * 512:(n + 1) * 512],
                                                 start=True, stop=False)
                                nc.tensor.matmul(pw[:], eT1[:, j:j + 1],
                                                 cn1[:, n * 512:(n + 1) * 512],
                                                 start=False, stop=True)
                                nc.any.tensor_copy(
                                    out=wsrow[:, n * 512:(n + 1) * 512], in_=pw[:])
                            nc.sync.dma_start(raw.ap()[b:b + 1, :], wsrow[:])

                    # ---- normalize group rows by 1/sumexpp ----
                    if "ws" in ABLATE:
                        continue
                    rawg = gpool.tile([GB, CONV_DIM], f32, tag="rawg")
                    nc.sync.dma_start(rawg[:], raw.ap()[g * GB:(g + 1) * GB, :])
                    outg = gpool.tile([GB, CONV_DIM], f32, tag="outg")
                    nc.scalar.mul(outg[:], rawg[:], rsum[:])
                    nc.sync.dma_start(out[g * GB:(g + 1) * GB, :], outg[:])

            if reps == 1:
                body()
            else:
                with tc.For_i(0, reps, 1):
                    body()

    nc.compile()
    return nc


def host_pack(conv_out, lstm_hidden, Wc, bc, Wl, bl, Wf, bf):
    """Host-side preprocessing -> (shared dict, per-core input dicts)."""
    conv_out = np.asarray(conv_out, dtype=np.float32)
    lstm_hidden = np.asarray(lstm_hidden, dtype=np.float32)
    Wc = np.asarray(Wc, dtype=np.float32)
    bc = np.asarray(bc, dtype=np.float32)
    Wl = np.asarray(Wl, dtype=np.float32)
    bl = np.asarray(bl, dtype=np.float32)
    wf = np.asarray(Wf, dtype=np.float32)[0]
    # permute a axis: Wf>0 first; fold |Wf| into Wc/Wl/bias
    pos = np.nonzero(wf > 0)[0]
    neg = np.nonzero(wf <= 0)[0]
    perm = np.concatenate([pos, neg])
    a_pos = int(len(pos))
    absf = np.abs(wf[perm])[:, None]                       # [512, 1]
    Wcp = (np.abs(wf)[:, None] * Wc)[perm]                 # [512, 2048]
    Wlp = (np.abs(wf)[:, None] * Wl)[perm]                 # [512, 512]
    biasp = (np.abs(wf) * (bc + bl))[perm]                 # [512]

    wc_pack = np.ascontiguousarray(
        Wcp.T.reshape(KO, 128, 512).transpose(1, 0, 2)).astype(DT1_NP)
    wl_pack = np.ascontiguousarray(
        Wlp.T.reshape(LK, 128, 512).transpose(1, 0, 2))
    shared = {
        "wc": wc_pack,
        "wl": wl_pack,
        "biasp": biasp[None, :].copy(),
        "onesb": np.ones((1, B), np.float32),
    }
    # indicator: ind[j, r] = 1 if r // 196 == j (r < RG)
    indm = np.zeros((GB, RGP), np.float32)
    r = np.arange(RG)
    indm[r // NPIX, r] = 1.0
    shared["ind"] = indm

    in_maps = []
    for c in range(N_CORES):
        sl = slice(c * B, (c + 1) * B)
        conv_core = conv_out[sl]                           # [32, 196, 2048]
        flat = conv_core.reshape(B * NPIX, CONV_DIM)
        ct = np.zeros((G, MC, 128, KO, 128), DT1_NP)
        for g in range(G):
            block = flat[g * RG:(g + 1) * RG]              # [1568, 2048]
            padded = np.zeros((RGP, CONV_DIM), np.float32)
            padded[:RG] = block
            # [mc, j, ko, p] -> [mc, p, ko, j]
            ct[g] = padded.reshape(MC, 128, KO, 128).transpose(0, 3, 2, 1)
        lstm_core = lstm_hidden[sl]                        # [32, 512]
        lstmT_pack = np.ascontiguousarray(
            lstm_core.T.reshape(LK, 128, B).transpose(1, 0, 2))
        in_maps.append(dict(shared,
                            convT=ct,
                            convN=np.ascontiguousarray(conv_core).astype(
                                np.float32 if WS_DT == f32r else ml_dtypes.bfloat16),
                            lstmT=lstmT_pack))
    return in_maps, a_pos


_CACHE = {}


def _get_program(a_pos, reps=1):
    key = (a_pos, reps)
    if key not in _CACHE:
        _CACHE[key] = build_program(a_pos, reps)
    return _CACHE[key]


def kernel(**inputs) -> np.ndarray:
    in_maps, a_pos = host_pack(**inputs)
    nc = _get_program(a_pos)
    res = bass_utils.run_bass_kernel_spmd(nc, in_maps,
                                          core_ids=list(range(N_CORES)))
    return np.concatenate([r["out"] for r in res.results], axis=0)
